# revision 5
# baseline (speedup 1.0000x reference)
"""Fused multi-head cross-attention with relation branch, sharded over 8 NeuronCores.

Sharding: data-parallel over batch (4) x tensor-parallel over head halves (2).
Core c handles batch c//2, heads [8*(c%2), 8*(c%2)+8). Each core computes its
partial output projection; the host sums the two partials per batch and adds bo.

Device data flow (per core, all matmuls fp32r):
  - q/k/rk projections emitted transposed: qT/kT/rkT [512 local dims, 1024 L]
    (4 chunks of 128 dims = head pairs (2dc, 2dc+1) at partitions 0-63/64-127)
  - v/rv projections emitted natural: [1024 LK, 512 dims], stored per lk-chunk
    with a ones column appended per head ([v_h | 1] of width 65) so the PV
    matmul's row 64 accumulates the softmax denominator for free.
  - scores computed transposed sT[lk, lq] = kT.T @ qT per head, two heads
    row-packed on the PE array (K=64 each at array rows 0-63 / 64-127).
  - exp + mask + 1/sqrt(dk) fused into one ACT op per score tile:
    p = exp(s*scale + bias[lk]) with bias = 0 / -1e9 from the key mask.
  - x_att^T accumulated in PSUM over lk chunks: [v_h|1].T @ p -> [65, lq].
  - normalization: recip of row 64, broadcast over 64 partitions via a rank-1
    PE matmul (ones[64] x recip), then DVE fma of the two branches.
  - output projection yT = WoT.T @ x_final accumulated over 4 dim chunks.
"""

import math

import numpy as np

B, LQ, LK, D, H = 4, 1024, 1024, 1024, 16
DK = D // H
SCALE = 1.0 / math.sqrt(DK)
N_CORES = 8
HD = D // 2  # local dims per core (8 heads * 64)

_CACHE = {}


def _build_program():
    import concourse.bacc as bacc
    import concourse.mybir as mybir
    import concourse.tile as tile

    f32 = mybir.dt.float32
    f32r = mybir.dt.float32r
    Exp = mybir.ActivationFunctionType.Exp
    Add = mybir.AluOpType.add
    Mult = mybir.AluOpType.mult

    nc = bacc.Bacc(
        "TRN2",
        target_bir_lowering=False,
        debug=False,
        enable_asserts=False,
        num_devices=N_CORES,
    )

    # DRAM I/O (per-core shapes; host shards/pre-transposes).
    xqT = nc.dram_tensor("xqT", [D, LQ], f32, kind="ExternalInput").ap()
    xkT = nc.dram_tensor("xkT", [D, LK], f32, kind="ExternalInput").ap()
    xrT = nc.dram_tensor("xrT", [D, LK], f32, kind="ExternalInput").ap()
    xvT = nc.dram_tensor("xvT", [D, LK], f32, kind="ExternalInput").ap()
    wqT = nc.dram_tensor("wqT", [D, HD], f32, kind="ExternalInput").ap()
    wkT = nc.dram_tensor("wkT", [D, HD], f32, kind="ExternalInput").ap()
    wrkT = nc.dram_tensor("wrkT", [D, HD], f32, kind="ExternalInput").ap()
    wvT = nc.dram_tensor("wvT", [D, HD], f32, kind="ExternalInput").ap()
    wrvT = nc.dram_tensor("wrvT", [D, HD], f32, kind="ExternalInput").ap()
    woT = nc.dram_tensor("woT", [HD, D], f32, kind="ExternalInput").ap()
    bq_pc = nc.dram_tensor("bq_pc", [128, 4], f32, kind="ExternalInput").ap()
    bk_pc = nc.dram_tensor("bk_pc", [128, 4], f32, kind="ExternalInput").ap()
    brk_pc = nc.dram_tensor("brk_pc", [128, 4], f32, kind="ExternalInput").ap()
    bv_bc = nc.dram_tensor("bv_bc", [128, HD], f32, kind="ExternalInput").ap()
    brv_bc = nc.dram_tensor("brv_bc", [128, HD], f32, kind="ExternalInput").ap()
    maskb = nc.dram_tensor("maskb", [128, 8], f32, kind="ExternalInput").ap()
    ones_d = nc.dram_tensor("ones_d", [128, 64], f32, kind="ExternalInput").ap()
    yT = nc.dram_tensor("yT", [D, LQ], f32, kind="ExternalOutput").ap()

    def r(ap):
        return ap.bitcast(f32r)

    with tile.TileContext(nc) as tc:
        from contextlib import ExitStack

        with ExitStack() as ctx:
            # Persistent SBUF tensors.
            persist = ctx.enter_context(tc.tile_pool(name="persist", bufs=1))
            qT_sb = persist.tile([128, 4 * LQ], f32, tag="qT")
            kT_sb = persist.tile([128, 4 * LK], f32, tag="kT")
            rkT_sb = persist.tile([128, 4 * LK], f32, tag="rkT")
            v_sb = persist.tile([128, 8 * 8 * 65], f32, tag="v")
            rv_sb = persist.tile([128, 8 * 8 * 65], f32, tag="rv")
            xf_sb = persist.tile([128, 4 * LQ], f32, tag="xf")
            ones_sb = persist.tile([128, 64], f32, tag="ones")
            maskb_sb = persist.tile([128, 8], f32, tag="maskb")
            bq_sb = persist.tile([128, 4], f32, tag="bq")
            bk_sb = persist.tile([128, 4], f32, tag="bk")
            brk_sb = persist.tile([128, 4], f32, tag="brk")
            bv_sb = persist.tile([128, HD], f32, tag="bv")
            brv_sb = persist.tile([128, HD], f32, tag="brv")

            nc.sync.dma_start(out=maskb_sb[:], in_=maskb)
            nc.sync.dma_start(out=bq_sb[:], in_=bq_pc)
            nc.sync.dma_start(out=bk_sb[:], in_=bk_pc)
            nc.sync.dma_start(out=brk_sb[:], in_=brk_pc)
            nc.sync.dma_start(out=bv_sb[:], in_=bv_bc)
            nc.sync.dma_start(out=brv_sb[:], in_=brv_bc)
            nc.sync.dma_start(out=ones_sb[:].bitcast(f32r), in_=ones_d.bitcast(f32r))

            v4 = v_sb[:].rearrange("p (m h c) -> p m h c", m=8, h=8, c=65)
            rv4 = rv_sb[:].rearrange("p (m h c) -> p m h c", m=8, h=8, c=65)
            ones_col = ones_d.rearrange("p (m h c) -> p m h c", m=8, h=8, c=1)
            nc.sync.dma_start(out=v4[:, :, :, 64:65].bitcast(f32r), in_=ones_col.bitcast(f32r))
            nc.sync.dma_start(out=rv4[:, :, :, 64:65].bitcast(f32r), in_=ones_col.bitcast(f32r))

            # ---------------- Phase 1: projections ----------------
            with ExitStack() as ph1:
                inp = ph1.enter_context(tc.tile_pool(name="inp", bufs=9))
                wblk = ph1.enter_context(tc.tile_pool(name="wblk", bufs=4))
                wmov = ph1.enter_context(tc.tile_pool(name="wmov", bufs=9))
                ppsum = ph1.enter_context(
                    tc.tile_pool(name="ppsum", bufs=2, space="PSUM")
                )

                # Transposed projections: out chunk dc = lhsT(Wt block).T @ x_chunk
                for name, xt, wt, b_sb, out_sb in (
                    ("q", xqT, wqT, bq_sb, qT_sb),
                    ("k", xkT, wkT, bk_sb, kT_sb),
                    ("rk", xrT, wrkT, brk_sb, rkT_sb),
                ):
                    xch = []
                    for k in range(8):
                        t = inp.tile([128, 1024], f32, tag="inp")
                        nc.sync.dma_start(
                            out=t[:].bitcast(f32r),
                            in_=xt[128 * k : 128 * k + 128, :].bitcast(f32r),
                        )
                        xch.append(t)
                    for dc in range(4):
                        ps = ppsum.tile([128, 1024], f32, tag="ppsum")
                        for k in range(8):
                            wb = wblk.tile([128, 128], f32, tag="wblk")
                            nc.sync.dma_start(
                                out=wb[:].bitcast(f32r),
                                in_=wt[
                                    128 * k : 128 * k + 128, 128 * dc : 128 * dc + 128
                                ].bitcast(f32r),
                            )
                            for lqh in range(2):
                                nc.tensor.matmul(
                                    ps[:, 512 * lqh : 512 * lqh + 512],
                                    r(wb[:]),
                                    r(xch[k][:, 512 * lqh : 512 * lqh + 512]),
                                    start=(k == 0),
                                    stop=(k == 7),
                                )
                        nc.vector.tensor_scalar(
                            out=out_sb[:, 1024 * dc : 1024 * dc + 1024].bitcast(f32r),
                            in0=ps[:],
                            scalar1=b_sb[:, dc : dc + 1],
                            scalar2=None,
                            op0=Add,
                        )

                # Natural-orientation projections for v / rv.
                for name, xt, wt, b_sb, out4 in (
                    ("v", xvT, wvT, bv_sb, v4),
                    ("rv", xrT, wrvT, brv_sb, rv4),
                ):
                    xch = []
                    for k in range(8):
                        t = inp.tile([128, 1024], f32, tag="inp")
                        nc.sync.dma_start(
                            out=t[:].bitcast(f32r),
                            in_=xt[128 * k : 128 * k + 128, :].bitcast(f32r),
                        )
                        xch.append(t)
                    wch = []
                    for k in range(8):
                        t = wmov.tile([128, HD], f32, tag="wmov")
                        nc.sync.dma_start(
                            out=t[:].bitcast(f32r),
                            in_=wt[128 * k : 128 * k + 128, :].bitcast(f32r),
                        )
                        wch.append(t)
                    for m in range(8):
                        ps = ppsum.tile([128, 512], f32, tag="ppsum")
                        for k in range(8):
                            nc.tensor.matmul(
                                ps[:],
                                r(xch[k][:, 128 * m : 128 * m + 128]),
                                r(wch[k][:]),
                                start=(k == 0),
                                stop=(k == 7),
                            )
                        nc.vector.tensor_tensor(
                            out=out4[:, m, :, 0:64].bitcast(f32r),
                            in0=ps[:].rearrange("p (h c) -> p h c", h=8, c=64),
                            in1=b_sb[:].rearrange("p (h c) -> p h c", h=8, c=64),
                            op=Add,
                        )

            # ---------------- Phase 2: attention ----------------
            with ExitStack() as ph2:
                spool = ph2.enter_context(tc.tile_pool(name="spool", bufs=2, space="PSUM"))
                xpool = ph2.enter_context(tc.tile_pool(name="xpool", bufs=4, space="PSUM"))
                ppool = ph2.enter_context(tc.tile_pool(name="ppool", bufs=3))
                xsb = ph2.enter_context(tc.tile_pool(name="xsb", bufs=6))
                rcp = ph2.enter_context(tc.tile_pool(name="rcp", bufs=2))

                for dc in range(4):
                    for lqh in range(2):
                        qsl = slice(1024 * dc + 512 * lqh, 1024 * dc + 512 * lqh + 512)
                        xacc = {}
                        for br in range(2):
                            for hs in range(2):
                                xacc[(br, hs)] = xpool.tile(
                                    [65, 512], f32, tag="xpool", name=f"xacc{br}{hs}"
                                )
                        for m in range(8):
                            ksl = slice(1024 * dc + 128 * m, 1024 * dc + 128 * m + 128)
                            pts = []
                            for br, kt in ((0, kT_sb), (1, rkT_sb)):
                                s = spool.tile([128, 1024], f32, tag="spool")
                                nc.tensor.matmul(
                                    s[:, 0:512], r(kt[0:64, ksl]), r(qT_sb[0:64, qsl])
                                )
                                nc.tensor.matmul(
                                    s[:, 512:1024],
                                    r(kt[64:128, ksl]),
                                    r(qT_sb[64:128, qsl]),
                                )
                                p = ppool.tile([128, 1024], f32, tag="ppool")
                                nc.scalar.activation(
                                    p[:].bitcast(f32r),
                                    s[:],
                                    Exp,
                                    bias=maskb_sb[:, m : m + 1],
                                    scale=SCALE,
                                )
                                pts.append(p)
                            for br, vv in ((0, v4), (1, rv4)):
                                for hs in range(2):
                                    nc.tensor.matmul(
                                        xacc[(br, hs)][:],
                                        r(vv[:, m, 2 * dc + hs, :]),
                                        r(pts[br][:, 512 * hs : 512 * hs + 512]),
                                        start=(m == 0),
                                        stop=(m == 7),
                                    )
                        # Normalalize and combine branches per head.
                        for hs in range(2):
                            xv_s = xsb.tile([65, 512], f32, tag="xsb")
                            nc.vector.tensor_copy(out=xv_s[:], in_=xacc[(0, hs)][:])
                            xr_s = xsb.tile([65, 512], f32, tag="xsb")
                            nc.vector.tensor_copy(out=xr_s[:], in_=xacc[(1, hs)][:])
                            rc = rcp.tile([65, 1024], f32, tag="rcp")
                            with nc.allow_low_precision(reason="fp32r recip feed"):
                                nc.vector.reciprocal(
                                    rc[64:65, 0:512].bitcast(f32r), xv_s[64:65, :]
                                )
                                nc.vector.reciprocal(
                                    rc[64:65, 512:1024].bitcast(f32r), xr_s[64:65, :]
                                )
                            bcv = spool.tile([64, 512], f32, tag="spool")
                            nc.tensor.matmul(
                                bcv[:], r(ones_sb[64:65, :]), r(rc[64:65, 0:512])
                            )
                            bcr = spool.tile([64, 512], f32, tag="spool")
                            nc.tensor.matmul(
                                bcr[:], r(ones_sb[64:65, :]), r(rc[64:65, 512:1024])
                            )
                            t1 = xsb.tile([65, 512], f32, tag="xsb")
                            nc.vector.tensor_tensor(
                                out=t1[0:64, :], in0=xv_s[0:64, :], in1=bcv[:], op=Mult
                            )
                            t2 = xsb.tile([65, 512], f32, tag="xsb")
                            nc.vector.tensor_tensor(
                                out=t2[0:64, :], in0=xr_s[0:64, :], in1=bcr[:], op=Mult
                            )
                            xf_slice = slice(
                                1024 * dc + 512 * lqh, 1024 * dc + 512 * lqh + 512
                            )
                            if hs == 0:
                                nc.vector.tensor_tensor(
                                    out=xf_sb[0:64, xf_slice].bitcast(f32r),
                                    in0=t1[0:64, :],
                                    in1=t2[0:64, :],
                                    op=Add,
                                )
                            else:
                                t3 = xsb.tile([65, 512], f32, tag="xsb")
                                nc.vector.tensor_tensor(
                                    out=t3[0:64, :].bitcast(f32r),
                                    in0=t1[0:64, :],
                                    in1=t2[0:64, :],
                                    op=Add,
                                )
                                nc.sync.dma_start(
                                    out=xf_sb[64:128, xf_slice].bitcast(f32r),
                                    in_=t3[0:64, :].bitcast(f32r),
                                )

            # ---------------- Phase 3: output projection ----------------
            with ExitStack() as ph3:
                opsum = ph3.enter_context(tc.tile_pool(name="opsum", bufs=4, space="PSUM"))
                wop = ph3.enter_context(tc.tile_pool(name="wop", bufs=4))
                ysb = ph3.enter_context(tc.tile_pool(name="ysb", bufs=4))

                for ot in range(8):
                    pss = [
                        opsum.tile([128, 512], f32, tag="opsum", name=f"psy{i}")
                        for i in range(2)
                    ]
                    for dc in range(4):
                        wo = wop.tile([128, 128], f32, tag="wop")
                        nc.sync.dma_start(
                            out=wo[:].bitcast(f32r),
                            in_=woT[
                                128 * dc : 128 * dc + 128, 128 * ot : 128 * ot + 128
                            ].bitcast(f32r),
                        )
                        for lqh in range(2):
                            nc.tensor.matmul(
                                pss[lqh][:],
                                r(wo[:]),
                                r(xf_sb[:, 1024 * dc + 512 * lqh : 1024 * dc + 512 * lqh + 512]),
                                start=(dc == 0),
                                stop=(dc == 3),
                            )
                    for lqh in range(2):
                        y = ysb.tile([128, 512], f32, tag="ysb")
                        nc.vector.tensor_copy(out=y[:], in_=pss[lqh][:])
                        nc.sync.dma_start(
                            out=yT[128 * ot : 128 * ot + 128, 512 * lqh : 512 * lqh + 512],
                            in_=y[:],
                        )

    nc.compile()
    return nc


def _get_program():
    if "nc" not in _CACHE:
        _CACHE["nc"] = _build_program()
    return _CACHE["nc"]


def _shard_inputs(inputs):
    q = np.ascontiguousarray(inputs["query"], dtype=np.float32)
    k = np.ascontiguousarray(inputs["key"], dtype=np.float32)
    v = np.ascontiguousarray(inputs["value"], dtype=np.float32)
    wr = np.ascontiguousarray(inputs["weak_rela"], dtype=np.float32)
    mask = np.asarray(inputs["mask"])

    in_maps = []
    for c in range(N_CORES):
        b, hh = divmod(c, 2)
        hsl = slice(HD * hh, HD * hh + HD)
        mb = np.where(
            mask[b, 0].reshape(8, 128).T.astype(bool), 0.0, -1.0e9
        ).astype(np.float32)
        m = {
            "xqT": q[b].T.copy(),
            "xkT": k[b].T.copy(),
            "xrT": wr[b].T.copy(),
            "xvT": v[b].T.copy(),
            "wqT": inputs["Wq"][hsl, :].T.copy(),
            "wkT": inputs["Wk"][hsl, :].T.copy(),
            "wrkT": inputs["Wrk"][hsl, :].T.copy(),
            "wvT": inputs["Wv"][hsl, :].T.copy(),
            "wrvT": inputs["Wrv"][hsl, :].T.copy(),
            "woT": inputs["Wo"][:, hsl].T.copy(),
            "bq_pc": inputs["bq"][hsl].reshape(4, 128).T.copy(),
            "bk_pc": inputs["bk"][hsl].reshape(4, 128).T.copy(),
            "brk_pc": inputs["brk"][hsl].reshape(4, 128).T.copy(),
            "bv_bc": np.broadcast_to(inputs["bv"][hsl], (128, HD)).copy(),
            "brv_bc": np.broadcast_to(inputs["brv"][hsl], (128, HD)).copy(),
            "maskb": mb,
            "ones_d": np.ones((128, 64), np.float32),
        }
        in_maps.append({k2: np.ascontiguousarray(v2, np.float32) for k2, v2 in m.items()})
    return in_maps


def run_on_hw(inputs, trace=False, **kw):
    from concourse.bass_utils import run_bass_kernel_spmd

    nc = _get_program()
    in_maps = _shard_inputs(inputs)
    res = run_bass_kernel_spmd(
        nc, in_maps, core_ids=list(range(N_CORES)), trace=trace, **kw
    )
    bo = np.asarray(inputs["bo"], dtype=np.float32)
    outs = []
    for b in range(B):
        yt = res.results[2 * b]["yT"] + res.results[2 * b + 1]["yT"]
        outs.append(yt.T + bo)
    out = np.stack(outs).astype(np.float32)
    return out, res


def kernel(**inputs):
    out, _ = run_on_hw(inputs)
    return out
